# revision 1
# baseline (speedup 1.0000x reference)
"""Trainium2 Bass kernel for nn_AggressiveLoss.

Strategy (pure data parallel, 8 NeuronCores):
  - B=1024 batch sharded 128 per core -> 128 SBUF partitions.
  - Free axis = [C=10, HW=900] f32.  All per-pixel C-reductions (argmax,
    log-softmax) are done with fp16 tensor ops (DVE 2x mode) via
    max/sum trees over the C slices; masks via is_equal against the
    broadcast max; selections (p[tgt_idx] etc.) via mask-multiply +
    sum tree (exact: one-hot).
  - argmax-equality tests between tensors a,b use  a_at_b == a_max
    where a_at_b = sum_c a_c * onehot_b_c; only eq_t (target) and
    eq_p (pred) one-hot masks are materialized.
  - exp/ln on ScalarE (f32-internal, dtype-independent rate).
  - Per-batch-element statistics accumulate free via
    scalar_tensor_tensor(accum_out=); per-color presence counts come from
    activation(Copy, accum_out=) on ScalarE at zero DVE cost.
  - Each core DMAs a [128, 24] f32 stat block out; the host combines
    all 1024 rows and evaluates the final scalar formula in float64.
"""

import os
import sys

sys.path.insert(0, "/opt/pypackages")
sys.path.insert(0, "/opt/trn_rl_repo")

import numpy as np

from concourse import bacc, mybir
from concourse import bass_utils
from concourse.tile import TileContext
from concourse.tile_rust import add_dep_helper
from concourse.mybir import AluOpType

F32 = mybir.dt.float32
F16 = mybir.dt.float16
ACT = mybir.ActivationFunctionType

B, C, HW = 1024, 10, 900
NCORES = 8
BL = B // NCORES  # 128 batch rows per core == SBUF partitions
NSTAT = 24  # per-b stats: ce_w, n_inc, eq_pi, eq_ti, pred_cnt[10], tgt_cnt[10]

_CACHED = {}


def _build():
    nc = bacc.Bacc(
        "TRN2",
        target_bir_lowering=False,
        debug=False,
        enable_asserts=False,
        num_devices=NCORES,
    )
    dp = nc.dram_tensor("pred", [BL, C, HW], F32, kind="ExternalInput").ap()
    dt = nc.dram_tensor("target", [BL, C, HW], F32, kind="ExternalInput").ap()
    di = nc.dram_tensor("input_grid", [BL, C, HW], F32, kind="ExternalInput").ap()
    dout = nc.dram_tensor("out", [BL, NSTAT], F32, kind="ExternalOutput").ap()

    with TileContext(nc) as tc:
        with (
            tc.tile_pool(name="stage", bufs=3) as stage_pool,
            tc.tile_pool(name="f16", bufs=1) as f16_pool,
            tc.tile_pool(name="eq", bufs=1) as eq_pool,
            tc.tile_pool(name="big", bufs=2) as big_pool,
            tc.tile_pool(name="scratch", bufs=1) as scr_pool,
            tc.tile_pool(name="small", bufs=1) as small_pool,
            tc.tile_pool(name="outp", bufs=1) as out_pool,
        ):
            # ~quarter splits with EVEN px offsets: fp16 slice starts must be
            # 4B-aligned or the DVE 2x perf mode silently drops to 1x on HW
            QB = (0, 226, 450, 676, 900)

            out32 = out_pool.tile([BL, NSTAT], F32, name="out32")

            # persistent fp16 copies
            t16 = f16_pool.tile([BL, C, HW], F16, name="t16")
            i16 = f16_pool.tile([BL, C, HW], F16, name="i16")
            p16 = f16_pool.tile([BL, C, HW], F16, name="p16")

            # stream in (t, p, i order): DMA f32 -> ACT cast fp16
            # (p second: the p-side has the most dependent DVE work; the
            #  i-side only needs imax + two products late in the schedule)
            # t lands in 6 slim pieces so the first DVE tree starts earliest.
            # piece layouts (px): cumulative offsets stay even so fp16 slices
            # remain 4B-aligned (2x-mode requirement)
            T_PIECES = (150, 150, 150, 150, 150, 150)
            QUARTERS = tuple(QB[k + 1] - QB[k] for k in range(4))
            sched = []
            for src, dst16, nm, pieces in (
                (dt, t16, "t", T_PIECES),
                (dp, p16, "p", QUARTERS),
                (di, i16, "i", QUARTERS),
            ):
                off = 0
                for h, W in enumerate(pieces):
                    sched.append((src, dst16, nm, h, off, W))
                    off += W
            order = list(range(14))
            last_cast = None
            for k in order:
                src, dst16, nm, h, off, W = sched[k]
                st = stage_pool.tile([BL, C, W], F32, name=f"st_{nm}{h}", tag="stage")
                nc.sync.dma_start(st[:], src[:, :, off : off + W])
                last_cast = nc.scalar.copy(dst16[:, :, off : off + W], st[:])

            mx = AluOpType.max
            add = AluOpType.add
            mul = AluOpType.mult
            eq = AluOpType.is_equal
            ne = AluOpType.not_equal
            TT = nc.vector.tensor_tensor

            # tree over the C axis (10 = 5+5 -> 2+2(+1) -> 1); px-range [lo:hi)
            def ctree(x, op, outt, nm, lo=0, hi=HW):
                s = (slice(None), slice(None), slice(lo, hi))
                w = hi - lo
                l5 = scr_pool.tile([BL, 5, HW], F16, name=f"l5_{nm}", tag="l5")
                l2 = scr_pool.tile([BL, 2, HW], F16, name=f"l2_{nm}", tag="l2")
                l1 = scr_pool.tile([BL, 1, HW], F16, name=f"l1_{nm}", tag="l1")
                r = (slice(None), slice(None), slice(0, w))
                TT(l5[:, :, lo:hi], x[:, 0:5, lo:hi], x[:, 5:10, lo:hi], op)
                TT(l2[:, :, lo:hi], l5[:, 0:2, lo:hi], l5[:, 2:4, lo:hi], op)
                TT(l1[:, :, lo:hi], l2[:, 0:1, lo:hi], l2[:, 1:2, lo:hi], op)
                TT(outt[:, :, lo:hi], l1[:, :, lo:hi], l5[:, 4:5, lo:hi], op)
                return outt

            # presence counts on ScalarE: activation(Copy) with accum_out
            # sums eq[:, c, :] over pixels per color -> zero DVE cost.
            pjunk = scr_pool.tile([BL, 1, HW], F16, name="pjunk", tag="pjunk")

            def presence(eqt, outcols, nm):
                for c in range(C):
                    inst = nc.scalar.activation(
                        pjunk[:],
                        eqt[:, c : c + 1, :],
                        ACT.Copy,
                        accum_out=outcols[:, c : c + 1],
                    )
                    # keep presence ops from preempting the input casts on ACT
                    add_dep_helper(inst.ins, last_cast.ins, sync=False)

            tmax = small_pool.tile([BL, 1, HW], F16, name="tmax")
            imax = small_pool.tile([BL, 1, HW], F16, name="imax")
            pmax = small_pool.tile([BL, 1, HW], F16, name="pmax")

            # --- target side: per-piece tm tree AND per-piece eq_t.  Each DMA
            # piece carries all C channels for its px range, so tmax (and thus
            # eq_t) for that range completes immediately — the eq work rides in
            # the DMA-cadence stalls where DVE op overhead is free. ---
            eq_t = eq_pool.tile([BL, C, HW], F16, name="eq_t")
            off = 0
            for q, W in enumerate(T_PIECES):
                ctree(t16, mx, tmax, f"tm{q}", off, off + W)
                TT(
                    eq_t[:, :, off : off + W],
                    t16[:, :, off : off + W],
                    tmax[:, :, off : off + W].broadcast_to([BL, C, W]),
                    eq,
                )
                off += W

            # --- pred side: per-quarter pm tree + eq_p (rides the p-DMA
            # stalls); prod1/s1 in halves interleaved so DVE chews on p-h0
            # while p-h1 still lands ---
            eq_p = eq_pool.tile([BL, C, HW], F16, name="eq_p")
            prod1 = big_pool.tile([BL, C, HW], F16, name="prod1", tag="big")
            p_at_t = small_pool.tile([BL, 1, HW], F16, name="p_at_t")

            def pq(k):
                ctree(p16, mx, pmax, f"pm{k}", QB[k], QB[k + 1])
                TT(
                    eq_p[:, :, QB[k] : QB[k + 1]],
                    p16[:, :, QB[k] : QB[k + 1]],
                    pmax[:, :, QB[k] : QB[k + 1]].broadcast_to(
                        [BL, C, QB[k + 1] - QB[k]]
                    ),
                    eq,
                )

            pq(0)
            pq(1)
            TT(
                prod1[:, :, 0 : HW // 2],
                p16[:, :, 0 : HW // 2],
                eq_t[:, :, 0 : HW // 2],
                mul,
            )
            ctree(prod1, add, p_at_t, "s1A", 0, HW // 2)
            pq(2)
            pq(3)
            TT(
                prod1[:, :, HW // 2 : HW],
                p16[:, :, HW // 2 : HW],
                eq_t[:, :, HW // 2 : HW],
                mul,
            )
            ctree(prod1, add, p_at_t, "s1B", HW // 2, HW)

            # --- selections ---
            i_at_p = small_pool.tile([BL, 1, HW], F16, name="i_at_p")
            i_at_t = small_pool.tile([BL, 1, HW], F16, name="i_at_t")

            # --- softmax pieces early: exp on ACT right after p casts, the
            # sum tree + Ln as soon as possible so the ce tail never stalls
            # exp in quarters so each piece can slot into ACT's DMA-wait gaps
            # without displacing a whole cast
            e16 = big_pool.tile([BL, C, HW], F16, name="e16", tag="big")
            for q in range(4):
                nc.scalar.activation(
                    e16[:, :, QB[q] : QB[q + 1]],
                    p16[:, :, QB[q] : QB[q + 1]],
                    ACT.Exp,
                )
            sum_e = small_pool.tile([BL, 1, HW], F16, name="sum_e", tag="w", bufs=3)
            ctree(e16, add, sum_e, "se")
            lse = small_pool.tile([BL, 1, HW], F16, name="lse", tag="w", bufs=3)
            nc.scalar.activation(lse[:], sum_e[:], ACT.Ln)

            # --- ce tail (needs only lse, p_at_t, pmax) ---
            ce = small_pool.tile([BL, 1, HW], F16, name="ce", tag="w", bufs=3)
            TT(ce[:], lse[:], p_at_t[:], AluOpType.subtract)
            # is_lt not not_equal: with an fp16 tie the one-hot mask goes
            # multi-hot and p_at_t = cnt*pmax; for pmax>0 that still compares
            # as a match, keeping exact-match detection right when pred==target
            # TT compare (2x) + ACT Copy-accum instead of 1x stt: this op is
            # off the critical end-chain, so the accum can ride on ScalarE
            inc = small_pool.tile([BL, 1, HW], F16, name="inc", tag="w", bufs=3)
            TT(inc[:], p_at_t[:], pmax[:], AluOpType.is_lt)
            pjunk2 = scr_pool.tile([BL, 1, HW], F16, name="pjunk2", tag="pjunk2")
            nc.scalar.activation(
                pjunk2[:], inc[:], ACT.Copy, accum_out=out32[:, 1:2]
            )
            tmp = small_pool.tile([BL, 1, HW], F16, name="tmp", tag="w", bufs=3)
            TT(tmp[:], ce[:], inc[:], mul)
            tmp4 = small_pool.tile([BL, 1, HW], F16, name="tmp4", tag="w", bufs=3)
            nc.vector.tensor_scalar_mul(tmp4[:], tmp[:], 4.0)
            ce_w = small_pool.tile([BL, 1, HW], F16, name="ce_w", tag="w", bufs=3)
            TT(ce_w[:], tmp4[:], ce[:], add)
            pjunk4 = scr_pool.tile([BL, 1, HW], F16, name="pjunk4", tag="pjunk4")
            nc.scalar.activation(
                pjunk4[:], ce_w[:], ACT.Copy, accum_out=out32[:, 0:1]
            )

            # --- input side: only the max (masks come from eq_t/eq_p) ---
            ctree(i16, mx, imax, "imA", 0, HW // 2)
            ctree(i16, mx, imax, "imB", HW // 2, HW)

            prod2 = big_pool.tile([BL, C, HW], F16, name="prod2", tag="big")
            TT(prod2[:, :, 0 : HW // 2], i16[:, :, 0 : HW // 2], eq_p[:, :, 0 : HW // 2], mul)
            ctree(prod2, add, i_at_p, "s2A", 0, HW // 2)
            TT(prod2[:, :, HW // 2 : HW], i16[:, :, HW // 2 : HW], eq_p[:, :, HW // 2 : HW], mul)
            ctree(prod2, add, i_at_p, "s2B", HW // 2, HW)
            eq_pi = small_pool.tile([BL, 1, HW], F16, name="eq_pi", tag="w", bufs=3)
            TT(eq_pi[:], i_at_p[:], imax[:], AluOpType.is_ge)
            pjunk3 = scr_pool.tile([BL, 1, HW], F16, name="pjunk3", tag="pjunk3")
            nc.scalar.activation(
                pjunk3[:], eq_pi[:], ACT.Copy, accum_out=out32[:, 2:3]
            )

            prod3 = big_pool.tile([BL, C, HW], F16, name="prod3", tag="big")
            TT(prod3[:], i16[:], eq_t[:], mul)
            ctree(prod3, add, i_at_t, "s3")
            eq_ti = small_pool.tile([BL, 1, HW], F16, name="eq_ti", tag="w", bufs=3)
            nc.vector.scalar_tensor_tensor(
                eq_ti[:],
                i_at_t[:],
                0.0,
                imax[:],
                add,
                AluOpType.is_ge,
                accum_out=out32[:, 3:4],
            )

            # presence counts late on ACT (emission order = ACT queue order;
            # these must not delay exp/Ln above)
            presence(eq_t, out32[:, 14:24], "t")
            presence(eq_p, out32[:, 4:14], "p")

            nc.sync.dma_start(dout[:], out32[:])

    nc.compile()
    return nc


def kernel(pred, target, input_grid):
    pred = np.ascontiguousarray(np.asarray(pred, dtype=np.float32))
    target = np.ascontiguousarray(np.asarray(target, dtype=np.float32))
    input_grid = np.ascontiguousarray(np.asarray(input_grid, dtype=np.float32))

    if "nc" not in _CACHED:
        _CACHED["nc"] = _build()
    nc = _CACHED["nc"]

    pr = pred.reshape(B, C, HW)
    tr = target.reshape(B, C, HW)
    ir = input_grid.reshape(B, C, HW)
    in_maps = [
        {
            "pred": pr[k * BL : (k + 1) * BL],
            "target": tr[k * BL : (k + 1) * BL],
            "input_grid": ir[k * BL : (k + 1) * BL],
        }
        for k in range(NCORES)
    ]
    res = bass_utils.run_bass_kernel_spmd(nc, in_maps, core_ids=list(range(NCORES)))
    stats = np.concatenate([r["out"] for r in res.results], axis=0)  # [1024, 24]
    return _host_combine(stats.astype(np.float64))


def _host_combine(s):
    npx = float(HW)
    ce_sum = s[:, 0]
    n_inc = s[:, 1]
    n_eq_pi = s[:, 2]
    n_eq_ti = s[:, 3]
    pred_present = s[:, 4:14] > 0.5
    tgt_present = s[:, 14:24] > 0.5

    ce_loss = ce_sum.sum() / (B * npx)
    exact = (n_inc == 0).astype(np.float64)
    exact_sum = exact.sum()
    exact_mean = exact_sum / B
    exact_bonus = -1.0 * exact_mean

    should_not_copy = (n_eq_ti < npx).astype(np.float64)
    did_copy = (n_eq_pi == npx).astype(np.float64)
    copy_penalty = 5.0 * np.mean(should_not_copy * did_copy)

    changed = (npx - n_eq_pi) / npx
    tgt_changed = (npx - n_eq_ti) / npx
    transform_diff = np.mean((changed - tgt_changed) ** 2)

    missing = np.sum(tgt_present & ~pred_present)
    color_penalty = 0.1 * float(missing)

    total = ce_loss + exact_bonus + copy_penalty + transform_diff + color_penalty
    if np.isnan(total):
        total = 2.0
    elif total > 100.0:
        total = 10.0
    f = np.float32
    return (
        f(total),
        f(ce_loss),
        f(copy_penalty),
        f(exact_mean),
        f(exact_sum),
        f(transform_diff),
    )


if __name__ == "__main__":
    rng = np.random.default_rng(0)
    outs = kernel(
        rng.standard_normal((B, C, 30, 30), dtype=np.float32),
        rng.standard_normal((B, C, 30, 30), dtype=np.float32),
        rng.standard_normal((B, C, 30, 30), dtype=np.float32),
    )
    print(outs)

